# revision 66
# baseline (speedup 1.0000x reference)
"""Multi-head self-attention (RoPE + diagonal mask) TRN2 Bass kernel, 8-core SPMD.

Sharding: core = batch*2 + head_half. Each core computes, for its batch and its
8 heads: QKV projection (fp16 matmuls, f32 PSUM), RoPE, transposed scores
S^T = K @ Q^T with the diagonal mask folded in as a -60000*I accumulate (exp
underflows to exactly 0), exp (no max-subtraction - scores are bounded), then a
*transposed* PV: the exp'd probabilities pt serve as
the matmul stationary ([128 kv, 128 q] chunks) with the augmented V (64 dims +
ones column) moving, so each 65-column stream fills all 128 PSUM rows - half
the PE time of the v-stationary formulation. The PV accumulators (8 q-chunks x
65 per q-half) share 2 PSUM banks via memset + start=False accumulation (a
start=True would zero the whole 2KB bank under every co-resident group).
Softmax denominators land in-psum as column 64 of each accumulator; in the
[q, vdim] layout normalization is a per-partition reciprocal + tensor_scalar
multiply (no DRAM broadcast dance). The normalized y is flipped back to
[hd, seq] with PE transpose matmuls through the aux PSUM pool, and the output
projection accumulates head-group pairs (t01 / t23) on-chip, halving the
output DMA. The two cores sharing a batch return partial projection outputs
which the host sums (tensor-parallel reduce).

PSUM budget (8 banks): score strips 2x[128,1024]f32 = 4, PV accumulator
[128,2,512]f32 = 2, aux (QKV accum / proj / transposes) 2x[128,512] = 2.
"""
import sys

sys.path.insert(0, "/opt/trn_rl_repo")

import numpy as np

import concourse.bass as bass
import concourse.mybir as mybir
import concourse.tile as tile
from concourse import bacc
from concourse.bass_utils import run_bass_kernel_spmd

FP16 = mybir.dt.float16
F32 = mybir.dt.float32

B = 4
S = 2048
DM = 1024
NH = 16
HD = 64
H_CORE = 8          # heads per core
N_CORES = 8
KT = DM // 128      # 8 k-tiles over the model dim
SC = S // 128       # 16 seq chunks of 128
SCALE = HD ** -0.5

SWAP_MASK = []
for _i in range(16):
    SWAP_MASK += [2 * _i + 1, 2 * _i]

_CACHE = {}


def _build_nc():
    nc = bacc.Bacc("TRN2", target_bir_lowering=False, debug=False, num_devices=N_CORES)

    xT_d = nc.dram_tensor("xT", [DM, S], FP16, kind="ExternalInput").ap()
    wq_d = nc.dram_tensor("wq", [DM, 512], FP16, kind="ExternalInput").ap()
    wk_d = nc.dram_tensor("wk", [DM, 512], FP16, kind="ExternalInput").ap()
    wv_d = nc.dram_tensor("wv", [DM, 512], FP16, kind="ExternalInput").ap()
    wp_d = nc.dram_tensor("wp", [512, DM], FP16, kind="ExternalInput").ap()
    cos_d = nc.dram_tensor("cosb", [128, S], FP16, kind="ExternalInput").ap()
    sin_d = nc.dram_tensor("sinb", [128, S], FP16, kind="ExternalInput").ap()
    neg_d = nc.dram_tensor("negi", [128, 128], FP16, kind="ExternalInput").ap()
    idn_d = nc.dram_tensor("ident", [128, 128], FP16, kind="ExternalInput").ap()
    out_d = [
        nc.dram_tensor(f"out{p}", [S, DM], F32, kind="ExternalOutput").ap()
        for p in range(2)
    ]

    Exp = mybir.ActivationFunctionType.Exp

    with tile.TileContext(nc) as tc:
        with (
            tc.tile_pool(name="consts", bufs=1) as consts,
            tc.tile_pool(name="phb", bufs=1) as phb,
            tc.tile_pool(name="rope", bufs=2) as ropep,
            tc.tile_pool(name="pt", bufs=9) as ptp,
            tc.tile_pool(name="yq", bufs=3) as yqp,
            tc.tile_pool(name="rz", bufs=2) as rzp,
            tc.tile_pool(name="outsb", bufs=4) as outp,
            tc.tile_pool(name="sps", bufs=2, space="PSUM") as spsp,
            tc.tile_pool(name="pvps", bufs=1, space="PSUM") as pvpsp,
            tc.tile_pool(name="aux", bufs=2, space="PSUM") as auxp,
        ):
            # ---- persistent tiles ----
            cos_sb = consts.tile([128, S], FP16)
            sin_sb = consts.tile([128, S], FP16)
            neg_sb = consts.tile([128, 128], FP16)
            idn_sb = consts.tile([128, 128], FP16)
            wp_sb = consts.tile([128, 4, DM], FP16)

            kT = [consts.tile([128, S], FP16, name=f"kT{t}", tag=f"kT{t}") for t in range(4)]
            qT = [consts.tile([128, S], FP16, name=f"qT{t}", tag=f"qT{t}") for t in range(4)]
            yn = [consts.tile([128, S], FP16, name=f"yn{t}", tag=f"yn{t}") for t in range(4)]
            v_sb = consts.tile([128, SC, H_CORE, HD + 1], FP16)
            nc.vector.memset(v_sb[:, :, :, HD:HD + 1], 1.0)

            # PV accumulator: 2 banks, 4 q-chunk slots of 65 per bank.
            pv = pvpsp.tile([128, 2, 512], F32, name="pv", tag="pv")

            # ---- inputs for the projections (released with phb) ----
            xT_sb = phb.tile([128, KT, S], FP16)
            wq_sb = phb.tile([128, KT, 512], FP16)
            wk_sb = phb.tile([128, KT, 512], FP16)
            wv_sb = phb.tile([128, KT, 512], FP16)
            _dma_engines = [nc.sync, nc.gpsimd, nc.scalar]
            # coarse gather-DMAs keep the HWDGE queues short; the first K/Q
            # weight and x column-block gathers are split into kt-halves across
            # queues so the prologue's first accumulation pieces start early
            def half_gather(eng, dst, src, half):
                ks = slice(4 * half, 4 * half + 4)
                eng.dma_start(
                    out=dst[:, ks, :],
                    in_=src.rearrange("(a p) f -> p a f", p=128)[:, ks, :])

            half_gather(nc.sync, wk_sb, wk_d, 0)
            half_gather(nc.gpsimd, xT_sb[:, :, 0:512], xT_d[:, 0:512], 0)
            half_gather(nc.scalar, wq_sb, wq_d, 0)
            nc.scalar.dma_start(out=cos_sb, in_=cos_d)
            half_gather(nc.sync, wk_sb, wk_d, 1)
            half_gather(nc.gpsimd, xT_sb[:, :, 0:512], xT_d[:, 0:512], 1)
            nc.gpsimd.dma_start(out=sin_sb, in_=sin_d)
            half_gather(nc.scalar, wq_sb, wq_d, 1)
            nc.scalar.dma_start(out=neg_sb, in_=neg_d)
            nc.scalar.dma_start(out=idn_sb, in_=idn_d)
            nc.sync.dma_start(
                out=xT_sb[:, :, 512:1024],
                in_=xT_d[:, 512:1024].rearrange("(a p) f -> p a f", p=128))
            nc.gpsimd.dma_start(
                out=wv_sb[:], in_=wv_d.rearrange("(a p) f -> p a f", p=128))
            nc.scalar.dma_start(
                out=xT_sb[:, :, 1024:1536],
                in_=xT_d[:, 1024:1536].rearrange("(a p) f -> p a f", p=128))
            nc.sync.dma_start(
                out=xT_sb[:, :, 1536:2048],
                in_=xT_d[:, 1536:2048].rearrange("(a p) f -> p a f", p=128))
            nc.gpsimd.dma_start(
                out=wp_sb[:], in_=wp_d.rearrange("(a p) f -> p a f", p=128))

            def accum512(dst_view, lhsT_of_kt, rhs_of_kt, name, src_rearrange=None,
                         width=512, evac_act=False):
                """8-step k-accumulation into a [128, width] aux psum, evac'd to dst."""
                ps = auxp.tile([128, width], F32, tag="aux", name=name)
                for kt in range(KT):
                    nc.tensor.matmul(
                        ps[:], lhsT_of_kt(kt), rhs_of_kt(kt),
                        start=(kt == 0), stop=(kt == KT - 1),
                    )
                src = ps[:] if src_rearrange is None else ps[:].rearrange(*src_rearrange, d=HD)
                if evac_act:
                    nc.scalar.copy(dst_view, src)
                else:
                    nc.vector.tensor_copy(dst_view, src)

            def emit_v(sc, pair, evac_act=False):
                """V for seq chunk sc, heads 2*pair..2*pair+1 (one t-group)."""
                accum512(
                    v_sb[:, sc, 2 * pair:2 * pair + 2, 0:HD],
                    lambda kt, sc=sc: xT_sb[:, kt, sc * 128:(sc + 1) * 128],
                    lambda kt, pair=pair: wv_sb[:, kt, pair * 128:(pair + 1) * 128],
                    name=f"vps{sc}_{pair}",
                    src_rearrange=("p (h d) -> p h d",),
                    width=128,
                    evac_act=evac_act,
                )

            rope_raw = {}

            def emit_kq_quarter(t, which, qc, evac_act=False, piece=None):
                """piece=None: whole 8-step accumulation; piece=0..3: split
                into four 2-matmul sub-groups (finer filler granularity)."""
                w_sb = wk_sb if which == 0 else wq_sb
                if qc == 0 and piece in (None, 0):
                    rope_raw[(t, which)] = ropep.tile(
                        [128, S], FP16, tag="raw", bufs=2, name=f"raw{t}_{which}")
                raw = rope_raw[(t, which)]
                key = (t, which, qc)
                if piece in (None, 0):
                    kq_ps[key] = auxp.tile(
                        [128, 512], F32, tag="aux", name=f"kq{t}_{which}_{qc}")
                ps = kq_ps[key]
                kts = range(KT) if piece is None else range(4 * piece, 4 * piece + 4)
                for kt in kts:
                    nc.tensor.matmul(
                        ps[:],
                        w_sb[:, kt, t * 128:(t + 1) * 128],
                        xT_sb[:, kt, qc * 512:(qc + 1) * 512],
                        start=(kt == 0), stop=(kt == KT - 1),
                        skip_group_check=True,
                    )
                if piece in (None, 1):
                    kq_ps.pop(key)
                    dst = raw[:, qc * 512:(qc + 1) * 512]
                    if evac_act:
                        nc.scalar.copy(dst, ps[:])
                    else:
                        nc.vector.tensor_copy(dst, ps[:])

            def emit_rope(t, which, c0=0, c1=S, last=True, mul_pool=False):
                raw = rope_raw[(t, which)]
                if last:
                    rope_raw.pop((t, which))
                dst = kT if which == 0 else qT
                cs = slice(c0, c1)
                sw = ropep.tile([128, c1 - c0], FP16, tag="sw", bufs=3,
                                name=f"sw{t}_{which}_{c0}")
                eng = nc.gpsimd if mul_pool else nc.vector
                nc.vector.stream_shuffle(sw[:], raw[:, cs], SWAP_MASK)
                eng.tensor_mul(raw[:, cs], raw[:, cs], cos_sb[:, cs])
                eng.tensor_mul(sw[:], sw[:], sin_sb[:, cs])
                eng.tensor_add(dst[t][:, cs], raw[:, cs], sw[:])

            def emit_transpose(t, par, qh, yq, name, split_evac=False):
                """PE-transpose the normalized [q, hd] chunks back to [hd, q]."""
                rows = slice(64 * par, 64 * par + 64)
                tp = auxp.tile([64, 8, 128], FP16, tag="aux", name=f"tp{name}")
                for qc in range(8):
                    nc.tensor.transpose(tp[:, qc, :], yq[:, qc, :], idn_sb[:])
                    if split_evac and qc == 3:
                        nc.vector.tensor_copy(
                            yn[t][rows, qh * 1024:qh * 1024 + 512],
                            tp[:, 0:4, :].rearrange("p a b -> p (a b)"))
                if split_evac:
                    nc.vector.tensor_copy(
                        yn[t][rows, qh * 1024 + 512:(qh + 1) * 1024],
                        tp[:, 4:8, :].rearrange("p a b -> p (a b)"))
                else:
                    nc.vector.tensor_copy(
                        yn[t][rows, qh * 1024:(qh + 1) * 1024],
                        tp[:].rearrange("p a b -> p (a b)"))

            proj_osb = {}

            def emit_proj(pair, sc, evac_act=False, nn_only=None):
                """Project head-groups t=2*pair,2*pair+1 for seq chunk sc, summed.
                nn_only splits the two output halves into separate calls."""
                key = (pair, sc)
                if nn_only in (None, 0):
                    proj_osb[key] = outp.tile(
                        [128, DM], F32, tag="osb", name=f"osb{pair}_{sc}")
                osb = proj_osb[key]
                nns = range(2) if nn_only is None else (nn_only,)
                for nn in nns:
                    pp = auxp.tile([128, 512], F32, tag="aux", name=f"pp{pair}_{sc}_{nn}")
                    for i, t in enumerate((2 * pair, 2 * pair + 1)):
                        nc.tensor.matmul(
                            pp[:],
                            yn[t][:, sc * 128:(sc + 1) * 128],
                            wp_sb[:, t, nn * 512:(nn + 1) * 512],
                            start=(i == 0),
                            stop=(i == 1),
                        )
                    use_act = (nn == 1) if evac_act == "alt" else evac_act
                    if use_act:
                        nc.scalar.copy(osb[:, nn * 512:(nn + 1) * 512], pp[:])
                    else:
                        nc.vector.tensor_copy(osb[:, nn * 512:(nn + 1) * 512], pp[:])
                    if evac_act == "alt":
                        _dma_engines[nn % 2].dma_start(
                            out=out_d[pair][sc * 128:(sc + 1) * 128,
                                            nn * 512:(nn + 1) * 512],
                            in_=osb[:, nn * 512:(nn + 1) * 512])
                if evac_act == "alt":
                    if nn_only in (None, 1):
                        proj_osb.pop(key)
                elif nn_only in (None, 1):
                    proj_osb.pop(key)
                    _dma_engines[sc % 2].dma_start(
                        out=out_d[pair][sc * 128:(sc + 1) * 128, :], in_=osb[:])

            # Fillers are (pe_cost_ns, fn, cont); drip() spends a fixed
            # PE-time budget per call so filler work spreads evenly across the
            # slots instead of bunching (which would delay the scores feeding
            # ACT). cont=True marks the second half of a split accumulation
            # group: FIFO order keeps any other aux-pool user behind it.
            fillers = []
            debt = [0.0]
            kq_ps = {}

            def drip(budget):
                debt[0] = min(debt[0] + budget, 1200.0)
                while fillers and debt[0] >= fillers[0][0]:
                    cost, fn, _ = fillers.pop(0)
                    debt[0] -= cost
                    fn()

            def push_front(cost, fn):
                i = 0
                while i < len(fillers) and fillers[i][2]:
                    i += 1
                fillers.insert(i, (cost, fn, False))

            KQH_COST, V_COST, PROJH_COST, TP_COST = 440.0, 440.0, 440.0, 460.0

            # ---- prologue: the minimum gating the first exp: the low half of
            # K and Q for tile 0 (covers scores kc<8, q-half 0); the rest
            # drips. Evacs ride the (idle) ACT engine - DVE is busy with rope ----
            for qc in range(2):
                for p in range(2):
                    emit_kq_quarter(0, 0, qc, evac_act=True, piece=p)
            emit_rope(0, 0, 0, 1024, last=False)
            for qc in range(2):
                for p in range(2):
                    emit_kq_quarter(0, 1, qc, evac_act=True, piece=p)
            emit_rope(0, 1, 0, 1024, last=False)

            def pv_slot(qc, w=65):
                return pv[:, qc // 4, (qc % 4) * 65:(qc % 4) * 65 + w]

            def emit_scores(t, rows, qh, kc, name):
                sps = spsp.tile([128, 1024], F32, tag="s", name=f"s{name}_{kc}")
                for qg in range(2):
                    q0 = qh * 1024 + qg * 512
                    nc.tensor.matmul(
                        sps[:, qg * 512:(qg + 1) * 512],
                        kT[t][rows, kc * 128:(kc + 1) * 128],
                        qT[t][rows, q0:q0 + 512],
                        start=True,
                        stop=True,
                    )
                return sps

            def kq_chunks(t):
                """Chunks of filler entries: each kq quarter is 2 cont-pieces;
                rope halves follow the pair of quarters they depend on."""
                out = []
                for which in (0, 1):
                    for qc in range(4):
                        out.append([
                            (KQH_COST,
                             lambda w=which, qc=qc, t=t, p=p: emit_kq_quarter(
                                 t, w, qc, piece=p),
                             p > 0)
                            for p in range(2)
                        ])
                        if qc == 1:
                            out.append([(0.0,
                                         lambda w=which, t=t: emit_rope(
                                             t, w, 0, 1024, last=False,
                                             mul_pool=True),
                                         False)])
                        elif qc == 3:
                            out.append([(0.0,
                                         lambda w=which, t=t: emit_rope(
                                             t, w, 1024, S, mul_pool=True),
                                         False)])
                return out

            def proj_entries(pair, scs):
                out = []
                for sc in scs:
                    out.append((PROJH_COST,
                                lambda pair=pair, sc=sc: emit_proj(pair, sc, nn_only=0),
                                False))
                    out.append((PROJH_COST,
                                lambda pair=pair, sc=sc: emit_proj(pair, sc, nn_only=1),
                                False))
                return out

            def interleave(a, b):
                """a: list of entry-chunks; b: list of single entries."""
                out = []
                for i in range(max(len(a), len(b))):
                    if i < len(b):
                        out.append(b[i])
                    if i < len(a):
                        out.extend(a[i])
                return out

            def pv8g(kc, pt, hh):
                for qc in range(8):
                    nc.tensor.matmul(
                        pv_slot(qc),
                        pt[:, qc * 128:(qc + 1) * 128],
                        v_sb[:, kc, hh, :],
                        start=False,
                        stop=True,
                        skip_group_check=True,
                    )

            def emit_normalize(ot, opar, oqh, oh, last):
                """Reciprocal + per-partition scale in [q, hd] layout, then the
                transpose back to [hd, q] (queued as a filler unless last)."""
                rz = rzp.tile([128, 8], F32, tag="rz", name=f"rz{oh}_{oqh}")
                pvap = pv[:]
                zs = bass.AP(
                    tensor=pvap.tensor,
                    offset=pvap.offset + 64,
                    ap=[list(pvap.ap[0]), [512, 2], [65, 4]],
                )
                nc.vector.reciprocal(rz[:].rearrange("p (a b) -> p a b", a=2), zs)
                yq = yqp.tile([128, 8, HD], FP16, tag="yq", name=f"yq{oh}_{oqh}")
                for qc in range(8):
                    nc.vector.tensor_scalar_mul(
                        yq[:, qc, :], pv_slot(qc, HD), rz[:, qc:qc + 1])
                if last:
                    emit_transpose(ot, opar, oqh, yq, f"{oh}_{oqh}",
                                   split_evac=True)
                else:
                    push_front(TP_COST,
                               lambda t=ot, par=opar, qh=oqh, yq=yq, h=oh:
                               emit_transpose(t, par, qh, yq, f"{h}_{qh}"))

            # ---- attention slots: (t, qh, par) ordering so that by the time
            # the last head's qh=1 half runs, proj for the earlier halves can
            # already drip. V for pair p and K/Q for tile t arrive one block
            # ahead of first use, spread across the preceding block's slots. ----
            slots = [(t, qh, par) for t in range(4) for qh in range(2)
                     for par in range(2)]
            handoff = {"sps": None}
            carry = {"ptq": [], "h": 0, "t": 0, "par": 0, "qh": 0}
            for t in range(4):
                for qh in range(2):
                    for par in range(2):
                        h = 2 * t + par
                        slot = 4 * t + 2 * qh + par
                        if slot == 0:
                            for w in (0, 1):
                                for qc in range(2, 4):
                                    for p in range(2):
                                        fillers.append(
                                            (KQH_COST,
                                             lambda w=w, qc=qc, p=p: emit_kq_quarter(
                                                 0, w, qc, piece=p),
                                             p > 0))
                                    fillers.append(
                                        (0.0,
                                         lambda w=w, qc=qc: emit_rope(
                                             0, w, qc * 512, (qc + 1) * 512,
                                             last=(qc == 3)),
                                         False))
                            fillers += interleave(
                                kq_chunks(1),
                                [(V_COST, lambda sc=sc: emit_v(sc, 1), False)
                                 for sc in range(SC)])
                        elif slot == 4:
                            fillers += interleave(
                                kq_chunks(2),
                                [(V_COST, lambda sc=sc: emit_v(sc, 2), False)
                                 for sc in range(SC)])
                        elif slot == 8:
                            fillers += interleave(
                                kq_chunks(3),
                                [(V_COST, lambda sc=sc: emit_v(sc, 3), False)
                                 for sc in range(SC)])
                            # t01 projections: heads 0-3 fully transposed by now
                            # (the slot-7 transposes are at the queue front)
                            fillers += proj_entries(0, range(SC))
                        elif slot == 14:
                            # t23 projections for the qh=0 seq half
                            fillers += proj_entries(1, range(8))

                        rows = slice(64 * par, 64 * par + 64)
                        if slot == 0:
                            # zero the PV accumulator regions (start=False
                            # accumulation; GPSIMD cannot touch PSUM, so DVE)
                            nc.vector.memset(pv[:, :, 0:260], 0.0)
                        # software pipeline: scores(kc+1) and any filler are
                        # issued before pv(kc), which lags its exp by one strip
                        # so it never head-of-line-blocks the PE queue. The
                        # first scores of this slot were already emitted at the
                        # tail of the previous slot (before its last pv8).
                        if handoff["sps"] is not None:
                            sps = handoff["sps"]
                            handoff["sps"] = None
                        else:
                            sps = emit_scores(t, rows, qh, 0, f"{h}_{qh}")
                            emit_v(0, 0, evac_act=True)
                            emit_v(1, 0, evac_act=True)
                        drip(800)

                        ptq = []
                        for kc in range(SC):
                            pt = ptp.tile([128, 1024], FP16, tag="pt", name=f"pt{h}_{qh}_{kc}")
                            nc.scalar.activation(pt[:], sps[:], Exp, scale=SCALE)
                            if kc // 8 == qh:
                                c0 = kc * 128 - qh * 1024
                                nc.vector.tensor_mul(
                                    pt[:, c0:c0 + 128], pt[:, c0:c0 + 128], neg_sb[:])
                            if kc + 1 < SC:
                                sps = emit_scores(t, rows, qh, kc + 1, f"{h}_{qh}")
                            if slot == 0 and kc + 2 < SC:
                                emit_v(kc + 2, 0)
                                drip(180)
                            else:
                                drip(270 if slot < 4 else 250)
                            # drain the previous slot's carried PV groups (2 per
                            # iteration), then its normalize + the accumulator
                            # re-zero; our own pv8s start at iteration 4 so the
                            # memset lands with two iterations of slack
                            if carry["ptq"]:
                                for _ in range(2):
                                    if carry["ptq"]:
                                        okc, opt = carry["ptq"].pop(0)
                                        pv8g(okc, opt, carry["h"])
                                if not carry["ptq"]:
                                    emit_normalize(carry["t"], carry["par"],
                                                   carry["qh"], carry["h"], False)
                                    nc.vector.memset(pv[:, :, 0:260], 0.0)
                            else:
                                ptq.append((kc, pt))
                                if len(ptq) > 4:
                                    okc, opt = ptq.pop(0)
                                    pv8g(okc, opt, h)
                                continue
                            ptq.append((kc, pt))
                        # pre-emit the next slot's first scores, then either
                        # carry the PV flush into the next slot or (last slot)
                        # flush and normalize here
                        if slot < 15:
                            nt, nqh, npar = slots[slot + 1]
                            handoff["sps"] = emit_scores(
                                nt, slice(64 * npar, 64 * npar + 64), nqh, 0,
                                f"{2 * nt + npar}_{nqh}")
                            carry.update(ptq=ptq, h=h, t=t, par=par, qh=qh)
                        else:
                            for okc, opt in ptq:
                                pv8g(okc, opt, h)
                            emit_normalize(t, par, qh, h, True)

            while fillers:
                fillers.pop(0)[1]()
            for sc in range(8, SC):
                emit_proj(1, sc, evac_act="alt")

    nc.compile()
    return nc


def _host_tables():
    theta = 1.0 / (10000.0 ** (np.arange(0, HD, 2, dtype=np.float32) / HD))
    ang = np.arange(S, dtype=np.float32)[:, None] * theta[None, :]  # [S, 32]
    cos = np.repeat(np.cos(ang).T, 2, axis=0)  # [64, S]
    sin_ = np.empty((HD, S), np.float32)
    sin_[0::2] = -np.sin(ang).T
    sin_[1::2] = np.sin(ang).T
    cosb = np.concatenate([cos, cos], axis=0).astype(np.float16)  # [128, S]
    sinb = np.concatenate([sin_, sin_], axis=0).astype(np.float16)
    negi = (1.0 - np.eye(128, dtype=np.float32)).astype(np.float16)
    ident = np.eye(128, dtype=np.float32).astype(np.float16)
    return cosb, sinb, negi, ident


def _in_maps(x, Wqkv, Wproj):
    cosb, sinb, negi, ident = _host_tables()
    maps = []
    for core in range(N_CORES):
        b, hh = divmod(core, 2)
        c0 = hh * 512
        maps.append(
            {
                "xT": np.ascontiguousarray(x[b].T).astype(np.float16),
                "wq": np.ascontiguousarray(Wqkv[:, c0:c0 + 512]).astype(np.float16),
                "wk": np.ascontiguousarray(Wqkv[:, DM + c0:DM + c0 + 512]).astype(np.float16),
                "wv": np.ascontiguousarray(Wqkv[:, 2 * DM + c0:2 * DM + c0 + 512]).astype(np.float16),
                "wp": np.ascontiguousarray(Wproj[c0:c0 + 512, :]).astype(np.float16),
                "cosb": cosb,
                "sinb": sinb,
                "negi": negi,
                "ident": ident,
            }
        )
    return maps


def kernel(x, Wqkv, Wproj):
    if "nc" not in _CACHE:
        _CACHE["nc"] = _build_nc()
    nc = _CACHE["nc"]

    x = np.asarray(x)
    Wqkv = np.asarray(Wqkv)
    Wproj = np.asarray(Wproj)

    res = run_bass_kernel_spmd(nc, _in_maps(x, Wqkv, Wproj), core_ids=list(range(N_CORES)))
    out = np.empty((B, S, DM), np.float32)
    for b in range(B):
        acc = None
        for core in (2 * b, 2 * b + 1):
            for p in range(2):
                part = res.results[core][f"out{p}"]
                acc = part if acc is None else acc + part
        out[b] = acc
    return out


# revision 73
# speedup vs baseline: 1.0005x; 1.0005x over previous
"""Multi-head self-attention (RoPE + diagonal mask) TRN2 Bass kernel, 8-core SPMD.

Sharding: core = batch*2 + head_half. Each core computes, for its batch and its
8 heads: QKV projection (fp16 matmuls, f32 PSUM), RoPE, transposed scores
S^T = K @ Q^T with the diagonal mask folded in as a -60000*I accumulate (exp
underflows to exactly 0), exp (no max-subtraction - scores are bounded), then a
*transposed* PV: the exp'd probabilities pt serve as
the matmul stationary ([128 kv, 128 q] chunks) with the augmented V (64 dims +
ones column) moving, so each 65-column stream fills all 128 PSUM rows - half
the PE time of the v-stationary formulation. The PV accumulators (8 q-chunks x
65 per q-half) share 2 PSUM banks via memset + start=False accumulation (a
start=True would zero the whole 2KB bank under every co-resident group).
Softmax denominators land in-psum as column 64 of each accumulator; in the
[q, vdim] layout normalization is a per-partition reciprocal + tensor_scalar
multiply (no DRAM broadcast dance). The normalized y is flipped back to
[hd, seq] with PE transpose matmuls through the aux PSUM pool, and the output
projection accumulates head-group pairs (t01 / t23) on-chip, halving the
output DMA. The two cores sharing a batch return partial projection outputs
which the host sums (tensor-parallel reduce).

PSUM budget (8 banks): score strips 2x[128,1024]f32 = 4, PV accumulator
[128,2,512]f32 = 2, aux (QKV accum / proj / transposes) 2x[128,512] = 2.
"""
import sys

sys.path.insert(0, "/opt/trn_rl_repo")

import numpy as np

import concourse.bass as bass
import concourse.mybir as mybir
import concourse.tile as tile
from concourse import bacc
from concourse.bass_utils import run_bass_kernel_spmd

FP16 = mybir.dt.float16
F32 = mybir.dt.float32

B = 4
S = 2048
DM = 1024
NH = 16
HD = 64
H_CORE = 8          # heads per core
N_CORES = 8
KT = DM // 128      # 8 k-tiles over the model dim
SC = S // 128       # 16 seq chunks of 128
SCALE = HD ** -0.5

SWAP_MASK = []
for _i in range(16):
    SWAP_MASK += [2 * _i + 1, 2 * _i]

_CACHE = {}


def _build_nc():
    nc = bacc.Bacc("TRN2", target_bir_lowering=False, debug=False, num_devices=N_CORES)

    xT_d = nc.dram_tensor("xT", [DM, S], FP16, kind="ExternalInput").ap()
    wq_d = nc.dram_tensor("wq", [DM, 512], FP16, kind="ExternalInput").ap()
    wk_d = nc.dram_tensor("wk", [DM, 512], FP16, kind="ExternalInput").ap()
    wv_d = nc.dram_tensor("wv", [DM, 512], FP16, kind="ExternalInput").ap()
    wp_d = nc.dram_tensor("wp", [512, DM], FP16, kind="ExternalInput").ap()
    cos_d = nc.dram_tensor("cosb", [128, S], FP16, kind="ExternalInput").ap()
    sin_d = nc.dram_tensor("sinb", [128, S], FP16, kind="ExternalInput").ap()
    neg_d = nc.dram_tensor("negi", [128, 128], FP16, kind="ExternalInput").ap()
    idn_d = nc.dram_tensor("ident", [128, 128], FP16, kind="ExternalInput").ap()
    out_d = [
        nc.dram_tensor(f"out{p}", [S, DM], F32, kind="ExternalOutput").ap()
        for p in range(2)
    ]

    Exp = mybir.ActivationFunctionType.Exp

    with tile.TileContext(nc) as tc:
        with (
            tc.tile_pool(name="consts", bufs=1) as consts,
            tc.tile_pool(name="phb", bufs=1) as phb,
            tc.tile_pool(name="rope", bufs=2) as ropep,
            tc.tile_pool(name="pt", bufs=9) as ptp,
            tc.tile_pool(name="yq", bufs=3) as yqp,
            tc.tile_pool(name="rz", bufs=2) as rzp,
            tc.tile_pool(name="outsb", bufs=4) as outp,
            tc.tile_pool(name="sps", bufs=2, space="PSUM") as spsp,
            tc.tile_pool(name="pvps", bufs=1, space="PSUM") as pvpsp,
            tc.tile_pool(name="aux", bufs=2, space="PSUM") as auxp,
        ):
            # ---- persistent tiles ----
            cos_sb = consts.tile([128, S], FP16)
            sin_sb = consts.tile([128, S], FP16)
            neg_sb = consts.tile([128, 128], FP16)
            idn_sb = consts.tile([128, 128], FP16)
            wp_sb = consts.tile([128, 4, DM], FP16)

            kT = [consts.tile([128, S], FP16, name=f"kT{t}", tag=f"kT{t}") for t in range(4)]
            qT = [consts.tile([128, S], FP16, name=f"qT{t}", tag=f"qT{t}") for t in range(4)]
            yn = [consts.tile([128, S], FP16, name=f"yn{t}", tag=f"yn{t}") for t in range(4)]
            v_sb = consts.tile([128, SC, H_CORE, HD + 1], FP16)
            nc.vector.memset(v_sb[:, :, :, HD:HD + 1], 1.0)

            # PV accumulator: 2 banks, 4 q-chunk slots of 65 per bank.
            pv = pvpsp.tile([128, 2, 512], F32, name="pv", tag="pv")

            # ---- inputs for the projections (released with phb) ----
            xT_sb = phb.tile([128, KT, S], FP16)
            wq_sb = phb.tile([128, KT, 512], FP16)
            wk_sb = phb.tile([128, KT, 512], FP16)
            wv_sb = phb.tile([128, KT, 512], FP16)
            _dma_engines = [nc.sync, nc.gpsimd, nc.scalar]
            # coarse gather-DMAs keep the HWDGE queues short; the first K/Q
            # weight and x column-block gathers are split into kt-halves across
            # queues so the prologue's first accumulation pieces start early
            def half_gather(eng, dst, src, half):
                ks = slice(4 * half, 4 * half + 4)
                eng.dma_start(
                    out=dst[:, ks, :],
                    in_=src.rearrange("(a p) f -> p a f", p=128)[:, ks, :])

            half_gather(nc.sync, wk_sb, wk_d, 0)
            half_gather(nc.gpsimd, xT_sb[:, :, 0:512], xT_d[:, 0:512], 0)
            half_gather(nc.scalar, wq_sb, wq_d, 0)
            nc.scalar.dma_start(out=cos_sb, in_=cos_d)
            half_gather(nc.sync, wk_sb, wk_d, 1)
            half_gather(nc.gpsimd, xT_sb[:, :, 0:512], xT_d[:, 0:512], 1)
            nc.gpsimd.dma_start(out=sin_sb, in_=sin_d)
            half_gather(nc.scalar, wq_sb, wq_d, 1)
            nc.scalar.dma_start(out=neg_sb, in_=neg_d)
            nc.scalar.dma_start(out=idn_sb, in_=idn_d)
            nc.sync.dma_start(
                out=xT_sb[:, :, 512:1024],
                in_=xT_d[:, 512:1024].rearrange("(a p) f -> p a f", p=128))
            nc.gpsimd.dma_start(
                out=wv_sb[:], in_=wv_d.rearrange("(a p) f -> p a f", p=128))
            nc.scalar.dma_start(
                out=xT_sb[:, :, 1024:1536],
                in_=xT_d[:, 1024:1536].rearrange("(a p) f -> p a f", p=128))
            nc.sync.dma_start(
                out=xT_sb[:, :, 1536:2048],
                in_=xT_d[:, 1536:2048].rearrange("(a p) f -> p a f", p=128))
            nc.gpsimd.dma_start(
                out=wp_sb[:], in_=wp_d.rearrange("(a p) f -> p a f", p=128))

            def accum512(dst_view, lhsT_of_kt, rhs_of_kt, name, src_rearrange=None,
                         width=512, evac_act=False):
                """8-step k-accumulation into a [128, width] aux psum, evac'd to dst."""
                ps = auxp.tile([128, width], F32, tag="aux", name=name)
                for kt in range(KT):
                    nc.tensor.matmul(
                        ps[:], lhsT_of_kt(kt), rhs_of_kt(kt),
                        start=(kt == 0), stop=(kt == KT - 1),
                    )
                src = ps[:] if src_rearrange is None else ps[:].rearrange(*src_rearrange, d=HD)
                if evac_act:
                    nc.scalar.copy(dst_view, src)
                else:
                    nc.vector.tensor_copy(dst_view, src)

            def emit_v(sc, pair, evac_act=False):
                """V for seq chunk sc, heads 2*pair..2*pair+1 (one t-group)."""
                accum512(
                    v_sb[:, sc, 2 * pair:2 * pair + 2, 0:HD],
                    lambda kt, sc=sc: xT_sb[:, kt, sc * 128:(sc + 1) * 128],
                    lambda kt, pair=pair: wv_sb[:, kt, pair * 128:(pair + 1) * 128],
                    name=f"vps{sc}_{pair}",
                    src_rearrange=("p (h d) -> p h d",),
                    width=128,
                    evac_act=evac_act,
                )

            rope_raw = {}

            def emit_kq_quarter(t, which, qc, evac_act=False, piece=None):
                """piece=None: whole 8-step accumulation; piece=0..3: split
                into four 2-matmul sub-groups (finer filler granularity)."""
                w_sb = wk_sb if which == 0 else wq_sb
                if qc == 0 and piece in (None, 0):
                    rope_raw[(t, which)] = ropep.tile(
                        [128, S], FP16, tag="raw", bufs=2, name=f"raw{t}_{which}")
                raw = rope_raw[(t, which)]
                key = (t, which, qc)
                if piece in (None, 0):
                    kq_ps[key] = auxp.tile(
                        [128, 512], F32, tag="aux", name=f"kq{t}_{which}_{qc}")
                ps = kq_ps[key]
                kts = range(KT) if piece is None else range(4 * piece, 4 * piece + 4)
                for kt in kts:
                    nc.tensor.matmul(
                        ps[:],
                        w_sb[:, kt, t * 128:(t + 1) * 128],
                        xT_sb[:, kt, qc * 512:(qc + 1) * 512],
                        start=(kt == 0), stop=(kt == KT - 1),
                        skip_group_check=True,
                    )
                if piece in (None, 1):
                    kq_ps.pop(key)
                    dst = raw[:, qc * 512:(qc + 1) * 512]
                    if evac_act:
                        nc.scalar.copy(dst, ps[:])
                    else:
                        nc.vector.tensor_copy(dst, ps[:])

            def emit_rope(t, which, c0=0, c1=S, last=True, mul_pool=False):
                raw = rope_raw[(t, which)]
                if last:
                    rope_raw.pop((t, which))
                dst = kT if which == 0 else qT
                cs = slice(c0, c1)
                sw = ropep.tile([128, c1 - c0], FP16, tag="sw", bufs=3,
                                name=f"sw{t}_{which}_{c0}")
                eng = nc.gpsimd if mul_pool else nc.vector
                nc.vector.stream_shuffle(sw[:], raw[:, cs], SWAP_MASK)
                eng.tensor_mul(raw[:, cs], raw[:, cs], cos_sb[:, cs])
                eng.tensor_mul(sw[:], sw[:], sin_sb[:, cs])
                eng.tensor_add(dst[t][:, cs], raw[:, cs], sw[:])

            def emit_transpose(t, par, qh, yq, name, split_evac=False):
                """PE-transpose the normalized [q, hd] chunks back to [hd, q]."""
                rows = slice(64 * par, 64 * par + 64)
                tp = auxp.tile([64, 8, 128], FP16, tag="aux", name=f"tp{name}")
                for qc in range(8):
                    nc.tensor.transpose(tp[:, qc, :], yq[:, qc, :], idn_sb[:])
                    if split_evac and qc == 3:
                        nc.vector.tensor_copy(
                            yn[t][rows, qh * 1024:qh * 1024 + 512],
                            tp[:, 0:4, :].rearrange("p a b -> p (a b)"))
                if split_evac:
                    nc.vector.tensor_copy(
                        yn[t][rows, qh * 1024 + 512:(qh + 1) * 1024],
                        tp[:, 4:8, :].rearrange("p a b -> p (a b)"))
                else:
                    nc.vector.tensor_copy(
                        yn[t][rows, qh * 1024:(qh + 1) * 1024],
                        tp[:].rearrange("p a b -> p (a b)"))

            proj_osb = {}

            def emit_proj(pair, sc, evac_act=False, nn_only=None):
                """Project head-groups t=2*pair,2*pair+1 for seq chunk sc, summed.
                nn_only splits the two output halves into separate calls."""
                key = (pair, sc)
                if nn_only in (None, 0):
                    proj_osb[key] = outp.tile(
                        [128, DM], F32, tag="osb", name=f"osb{pair}_{sc}")
                osb = proj_osb[key]
                nns = range(2) if nn_only is None else (nn_only,)
                for nn in nns:
                    pp = auxp.tile([128, 512], F32, tag="aux", name=f"pp{pair}_{sc}_{nn}")
                    for i, t in enumerate((2 * pair, 2 * pair + 1)):
                        nc.tensor.matmul(
                            pp[:],
                            yn[t][:, sc * 128:(sc + 1) * 128],
                            wp_sb[:, t, nn * 512:(nn + 1) * 512],
                            start=(i == 0),
                            stop=(i == 1),
                        )
                    use_act = (nn == 1) if evac_act == "alt" else evac_act
                    if use_act:
                        nc.scalar.copy(osb[:, nn * 512:(nn + 1) * 512], pp[:])
                    else:
                        nc.vector.tensor_copy(osb[:, nn * 512:(nn + 1) * 512], pp[:])
                    if evac_act == "alt":
                        _dma_engines[nn % 2].dma_start(
                            out=out_d[pair][sc * 128:(sc + 1) * 128,
                                            nn * 512:(nn + 1) * 512],
                            in_=osb[:, nn * 512:(nn + 1) * 512])
                if evac_act == "alt":
                    if nn_only in (None, 1):
                        proj_osb.pop(key)
                elif nn_only in (None, 1):
                    proj_osb.pop(key)
                    _dma_engines[sc % 2].dma_start(
                        out=out_d[pair][sc * 128:(sc + 1) * 128, :], in_=osb[:])

            # Fillers are (pe_cost_ns, fn, cont); drip() spends a fixed
            # PE-time budget per call so filler work spreads evenly across the
            # slots instead of bunching (which would delay the scores feeding
            # ACT). cont=True marks the second half of a split accumulation
            # group: FIFO order keeps any other aux-pool user behind it.
            fillers = []
            debt = [0.0]
            kq_ps = {}

            def drip(budget):
                debt[0] = min(debt[0] + budget, 1200.0)
                while fillers and debt[0] >= fillers[0][0]:
                    cost, fn, _ = fillers.pop(0)
                    debt[0] -= cost
                    fn()

            def push_front(cost, fn):
                i = 0
                while i < len(fillers) and fillers[i][2]:
                    i += 1
                fillers.insert(i, (cost, fn, False))

            KQH_COST, V_COST, PROJH_COST, TP_COST = 440.0, 440.0, 440.0, 460.0

            # ---- prologue: the minimum gating the first exp: the low half of
            # K and Q for tile 0 (covers scores kc<8, q-half 0); the rest
            # drips. Evacs ride the (idle) ACT engine - DVE is busy with rope ----
            for qc in range(2):
                for p in range(2):
                    emit_kq_quarter(0, 0, qc, evac_act=True, piece=p)
            emit_rope(0, 0, 0, 1024, last=False)
            for qc in range(2):
                for p in range(2):
                    emit_kq_quarter(0, 1, qc, evac_act=True, piece=p)
            emit_rope(0, 1, 0, 1024, last=False)

            def pv_slot(qc, w=65):
                return pv[:, qc // 4, (qc % 4) * 65:(qc % 4) * 65 + w]

            def emit_scores(t, rows, qh, kc, name):
                sps = spsp.tile([128, 1024], F32, tag="s", name=f"s{name}_{kc}")
                for qg in range(2):
                    q0 = qh * 1024 + qg * 512
                    nc.tensor.matmul(
                        sps[:, qg * 512:(qg + 1) * 512],
                        kT[t][rows, kc * 128:(kc + 1) * 128],
                        qT[t][rows, q0:q0 + 512],
                        start=True,
                        stop=True,
                    )
                return sps

            def kq_chunks(t):
                """Chunks of filler entries: each kq quarter is 2 cont-pieces;
                rope halves follow the pair of quarters they depend on."""
                out = []
                for which in (0, 1):
                    for qc in range(4):
                        out.append([
                            (KQH_COST,
                             lambda w=which, qc=qc, t=t, p=p: emit_kq_quarter(
                                 t, w, qc, piece=p),
                             p > 0)
                            for p in range(2)
                        ])
                        if qc == 1:
                            out.append([(0.0,
                                         lambda w=which, t=t: emit_rope(
                                             t, w, 0, 1024, last=False,
                                             mul_pool=True),
                                         False)])
                        elif qc == 3:
                            out.append([(0.0,
                                         lambda w=which, t=t: emit_rope(
                                             t, w, 1024, S, mul_pool=True),
                                         False)])
                return out

            def proj_entries(pair, scs):
                out = []
                for sc in scs:
                    out.append((PROJH_COST,
                                lambda pair=pair, sc=sc: emit_proj(pair, sc, nn_only=0),
                                False))
                    out.append((PROJH_COST,
                                lambda pair=pair, sc=sc: emit_proj(pair, sc, nn_only=1),
                                False))
                return out

            def interleave(a, b):
                """a: list of entry-chunks; b: list of single entries."""
                out = []
                for i in range(max(len(a), len(b))):
                    if i < len(b):
                        out.append(b[i])
                    if i < len(a):
                        out.extend(a[i])
                return out

            def pv8g(kc, pt, hh):
                for qc in range(8):
                    nc.tensor.matmul(
                        pv_slot(qc),
                        pt[:, qc * 128:(qc + 1) * 128],
                        v_sb[:, kc, hh, :],
                        start=False,
                        stop=True,
                        skip_group_check=True,
                    )

            def emit_normalize(ot, opar, oqh, oh, last):
                """Reciprocal + per-partition scale in [q, hd] layout, then the
                transpose back to [hd, q] (queued as a filler unless last)."""
                rz = rzp.tile([128, 8], F32, tag="rz", name=f"rz{oh}_{oqh}")
                pvap = pv[:]
                zs = bass.AP(
                    tensor=pvap.tensor,
                    offset=pvap.offset + 64,
                    ap=[list(pvap.ap[0]), [512, 2], [65, 4]],
                )
                nc.vector.reciprocal(rz[:].rearrange("p (a b) -> p a b", a=2), zs)
                yq = yqp.tile([128, 8, HD], FP16, tag="yq", name=f"yq{oh}_{oqh}")
                for qc in range(8):
                    if last and qc % 2 == 1:
                        # tail only: the exp stream is over, ACT is free -
                        # halve the serial normalize chain on the critical path
                        nc.scalar.mul(yq[:, qc, :], pv_slot(qc, HD),
                                      rz[:, qc:qc + 1])
                    else:
                        nc.vector.tensor_scalar_mul(
                            yq[:, qc, :], pv_slot(qc, HD), rz[:, qc:qc + 1])
                if last:
                    emit_transpose(ot, opar, oqh, yq, f"{oh}_{oqh}",
                                   split_evac=True)
                else:
                    push_front(TP_COST,
                               lambda t=ot, par=opar, qh=oqh, yq=yq, h=oh:
                               emit_transpose(t, par, qh, yq, f"{h}_{qh}"))

            # ---- attention slots: (t, qh, par) ordering so that by the time
            # the last head's qh=1 half runs, proj for the earlier halves can
            # already drip. V for pair p and K/Q for tile t arrive one block
            # ahead of first use, spread across the preceding block's slots. ----
            slots = [(t, qh, par) for t in range(4) for qh in range(2)
                     for par in range(2)]
            handoff = {"sps": None}
            carry = {"ptq": [], "h": 0, "t": 0, "par": 0, "qh": 0}
            for t in range(4):
                for qh in range(2):
                    for par in range(2):
                        h = 2 * t + par
                        slot = 4 * t + 2 * qh + par
                        if slot == 0:
                            for w in (0, 1):
                                for qc in range(2, 4):
                                    for p in range(2):
                                        fillers.append(
                                            (KQH_COST,
                                             lambda w=w, qc=qc, p=p: emit_kq_quarter(
                                                 0, w, qc, piece=p),
                                             p > 0))
                                    fillers.append(
                                        (0.0,
                                         lambda w=w, qc=qc: emit_rope(
                                             0, w, qc * 512, (qc + 1) * 512,
                                             last=(qc == 3)),
                                         False))
                            fillers += interleave(
                                kq_chunks(1),
                                [(V_COST, lambda sc=sc: emit_v(sc, 1), False)
                                 for sc in range(SC)])
                        elif slot == 4:
                            fillers += interleave(
                                kq_chunks(2),
                                [(V_COST, lambda sc=sc: emit_v(sc, 2), False)
                                 for sc in range(SC)])
                        elif slot == 8:
                            fillers += interleave(
                                kq_chunks(3),
                                [(V_COST, lambda sc=sc: emit_v(sc, 3), False)
                                 for sc in range(SC)])
                            # t01 projections: heads 0-3 fully transposed by now
                            # (the slot-7 transposes are at the queue front)
                            fillers += proj_entries(0, range(SC))
                        elif slot == 14:
                            # t23 projections for the qh=0 seq half
                            fillers += proj_entries(1, range(8))

                        rows = slice(64 * par, 64 * par + 64)
                        if slot == 0:
                            # zero the PV accumulator regions (start=False
                            # accumulation; GPSIMD cannot touch PSUM, so DVE)
                            nc.vector.memset(pv[:, :, 0:260], 0.0)
                        # software pipeline: scores(kc+1) and any filler are
                        # issued before pv(kc), which lags its exp by one strip
                        # so it never head-of-line-blocks the PE queue. The
                        # first scores of this slot were already emitted at the
                        # tail of the previous slot (before its last pv8).
                        if handoff["sps"] is not None:
                            sps = handoff["sps"]
                            handoff["sps"] = None
                        else:
                            sps = emit_scores(t, rows, qh, 0, f"{h}_{qh}")
                            emit_v(0, 0, evac_act=True)
                            emit_v(1, 0, evac_act=True)
                        drip(800)

                        ptq = []
                        for kc in range(SC):
                            pt = ptp.tile([128, 1024], FP16, tag="pt", name=f"pt{h}_{qh}_{kc}")
                            nc.scalar.activation(pt[:], sps[:], Exp, scale=SCALE)
                            if kc // 8 == qh:
                                c0 = kc * 128 - qh * 1024
                                nc.vector.tensor_mul(
                                    pt[:, c0:c0 + 128], pt[:, c0:c0 + 128], neg_sb[:])
                            if kc + 1 < SC:
                                sps = emit_scores(t, rows, qh, kc + 1, f"{h}_{qh}")
                            if slot == 0 and kc + 2 < SC:
                                emit_v(kc + 2, 0)
                                drip(180)
                            else:
                                drip(270 if slot < 4 else 250)
                            # drain the previous slot's carried PV groups (2 per
                            # iteration), then its normalize + the accumulator
                            # re-zero; our own pv8s start at iteration 4 so the
                            # memset lands with two iterations of slack
                            if carry["ptq"]:
                                for _ in range(2):
                                    if carry["ptq"]:
                                        okc, opt = carry["ptq"].pop(0)
                                        pv8g(okc, opt, carry["h"])
                                if not carry["ptq"]:
                                    emit_normalize(carry["t"], carry["par"],
                                                   carry["qh"], carry["h"], False)
                                    nc.vector.memset(pv[:, :, 0:260], 0.0)
                            else:
                                ptq.append((kc, pt))
                                if len(ptq) > 4:
                                    okc, opt = ptq.pop(0)
                                    pv8g(okc, opt, h)
                                continue
                            ptq.append((kc, pt))
                        # pre-emit the next slot's first scores, then either
                        # carry the PV flush into the next slot or (last slot)
                        # flush and normalize here
                        if slot < 15:
                            nt, nqh, npar = slots[slot + 1]
                            handoff["sps"] = emit_scores(
                                nt, slice(64 * npar, 64 * npar + 64), nqh, 0,
                                f"{2 * nt + npar}_{nqh}")
                            carry.update(ptq=ptq, h=h, t=t, par=par, qh=qh)
                        else:
                            for okc, opt in ptq:
                                pv8g(okc, opt, h)
                            emit_normalize(t, par, qh, h, True)

            while fillers:
                fillers.pop(0)[1]()
            for sc in range(8, SC):
                emit_proj(1, sc, evac_act="alt")

    nc.compile()
    return nc


def _host_tables():
    theta = 1.0 / (10000.0 ** (np.arange(0, HD, 2, dtype=np.float32) / HD))
    ang = np.arange(S, dtype=np.float32)[:, None] * theta[None, :]  # [S, 32]
    cos = np.repeat(np.cos(ang).T, 2, axis=0)  # [64, S]
    sin_ = np.empty((HD, S), np.float32)
    sin_[0::2] = -np.sin(ang).T
    sin_[1::2] = np.sin(ang).T
    cosb = np.concatenate([cos, cos], axis=0).astype(np.float16)  # [128, S]
    sinb = np.concatenate([sin_, sin_], axis=0).astype(np.float16)
    negi = (1.0 - np.eye(128, dtype=np.float32)).astype(np.float16)
    ident = np.eye(128, dtype=np.float32).astype(np.float16)
    return cosb, sinb, negi, ident


def _in_maps(x, Wqkv, Wproj):
    cosb, sinb, negi, ident = _host_tables()
    maps = []
    for core in range(N_CORES):
        b, hh = divmod(core, 2)
        c0 = hh * 512
        maps.append(
            {
                "xT": np.ascontiguousarray(x[b].T).astype(np.float16),
                "wq": np.ascontiguousarray(Wqkv[:, c0:c0 + 512]).astype(np.float16),
                "wk": np.ascontiguousarray(Wqkv[:, DM + c0:DM + c0 + 512]).astype(np.float16),
                "wv": np.ascontiguousarray(Wqkv[:, 2 * DM + c0:2 * DM + c0 + 512]).astype(np.float16),
                "wp": np.ascontiguousarray(Wproj[c0:c0 + 512, :]).astype(np.float16),
                "cosb": cosb,
                "sinb": sinb,
                "negi": negi,
                "ident": ident,
            }
        )
    return maps


def kernel(x, Wqkv, Wproj):
    if "nc" not in _CACHE:
        _CACHE["nc"] = _build_nc()
    nc = _CACHE["nc"]

    x = np.asarray(x)
    Wqkv = np.asarray(Wqkv)
    Wproj = np.asarray(Wproj)

    res = run_bass_kernel_spmd(nc, _in_maps(x, Wqkv, Wproj), core_ids=list(range(N_CORES)))
    out = np.empty((B, S, DM), np.float32)
    for b in range(B):
        acc = None
        for core in (2 * b, 2 * b + 1):
            for p in range(2):
                part = res.results[core][f"out{p}"]
                acc = part if acc is None else acc + part
        out[b] = acc
    return out


# revision 76
# speedup vs baseline: 1.0338x; 1.0333x over previous
"""Multi-head self-attention (RoPE + diagonal mask) TRN2 Bass kernel, 8-core SPMD.

Sharding: core = batch*2 + head_half. Each core computes, for its batch and its
8 heads: QKV projection (fp16 matmuls, f32 PSUM), RoPE, transposed scores
S^T = K @ Q^T with the diagonal mask folded in as a -60000*I accumulate (exp
underflows to exactly 0), exp (no max-subtraction - scores are bounded), then a
*transposed* PV: the exp'd probabilities pt serve as
the matmul stationary ([128 kv, 128 q] chunks) with the augmented V (64 dims +
ones column) moving, so each 65-column stream fills all 128 PSUM rows - half
the PE time of the v-stationary formulation. The PV accumulators (8 q-chunks x
65 per q-half) share 2 PSUM banks via memset + start=False accumulation (a
start=True would zero the whole 2KB bank under every co-resident group).
Softmax denominators land in-psum as column 64 of each accumulator; in the
[q, vdim] layout normalization is a per-partition reciprocal + tensor_scalar
multiply (no DRAM broadcast dance). The normalized y is flipped back to
[hd, seq] with PE transpose matmuls through the aux PSUM pool, and the output
projection accumulates head-group pairs (t01 / t23) on-chip, halving the
output DMA. The two cores sharing a batch return partial projection outputs
which the host sums (tensor-parallel reduce).

PSUM budget (8 banks): score strips 2x[128,1024]f32 = 4, PV accumulator
[128,2,512]f32 = 2, aux (QKV accum / proj / transposes) 2x[128,512] = 2.
"""
import sys

sys.path.insert(0, "/opt/trn_rl_repo")

import numpy as np

import concourse.bass as bass
import concourse.mybir as mybir
import concourse.tile as tile
from concourse import bacc
from concourse.bass_utils import run_bass_kernel_spmd

FP16 = mybir.dt.float16
F32 = mybir.dt.float32

B = 4
S = 2048
DM = 1024
NH = 16
HD = 64
H_CORE = 8          # heads per core
N_CORES = 8
KT = DM // 128      # 8 k-tiles over the model dim
SC = S // 128       # 16 seq chunks of 128
SCALE = HD ** -0.5

SWAP_MASK = []
for _i in range(16):
    SWAP_MASK += [2 * _i + 1, 2 * _i]

_CACHE = {}


def _build_nc():
    nc = bacc.Bacc("TRN2", target_bir_lowering=False, debug=False, num_devices=N_CORES)

    xT_d = nc.dram_tensor("xT", [DM, S], FP16, kind="ExternalInput").ap()
    wq_d = nc.dram_tensor("wq", [DM, 512], FP16, kind="ExternalInput").ap()
    wk_d = nc.dram_tensor("wk", [DM, 512], FP16, kind="ExternalInput").ap()
    wv_d = nc.dram_tensor("wv", [DM, 512], FP16, kind="ExternalInput").ap()
    wp_d = nc.dram_tensor("wp", [512, DM], FP16, kind="ExternalInput").ap()
    cos_d = nc.dram_tensor("cosb", [128, S], FP16, kind="ExternalInput").ap()
    sin_d = nc.dram_tensor("sinb", [128, S], FP16, kind="ExternalInput").ap()
    neg_d = nc.dram_tensor("negi", [128, 128], FP16, kind="ExternalInput").ap()
    idn_d = nc.dram_tensor("ident", [128, 128], FP16, kind="ExternalInput").ap()
    out_d = [
        nc.dram_tensor(f"out{p}", [S, DM], F32, kind="ExternalOutput").ap()
        for p in range(2)
    ]

    Exp = mybir.ActivationFunctionType.Exp

    with tile.TileContext(nc) as tc:
        with (
            tc.tile_pool(name="consts", bufs=1) as consts,
            tc.tile_pool(name="phb", bufs=1) as phb,
            tc.tile_pool(name="rope", bufs=2) as ropep,
            tc.tile_pool(name="pt", bufs=9) as ptp,
            tc.tile_pool(name="yq", bufs=3) as yqp,
            tc.tile_pool(name="rz", bufs=2) as rzp,
            tc.tile_pool(name="outsb", bufs=4) as outp,
            tc.tile_pool(name="sps", bufs=2, space="PSUM") as spsp,
            tc.tile_pool(name="pvps", bufs=1, space="PSUM") as pvpsp,
            tc.tile_pool(name="aux", bufs=2, space="PSUM") as auxp,
        ):
            # ---- persistent tiles ----
            cos_sb = consts.tile([128, S], FP16)
            sin_sb = consts.tile([128, S], FP16)
            neg_sb = consts.tile([128, 128], FP16)
            idn_sb = consts.tile([128, 128], FP16)
            wp_sb = consts.tile([128, 4, DM], FP16)

            kT = [consts.tile([128, S], FP16, name=f"kT{t}", tag=f"kT{t}") for t in range(4)]
            qT = [consts.tile([128, S], FP16, name=f"qT{t}", tag=f"qT{t}") for t in range(4)]
            yn = [consts.tile([128, S], FP16, name=f"yn{t}", tag=f"yn{t}") for t in range(4)]
            v_sb = consts.tile([128, SC, H_CORE, HD + 1], FP16)
            nc.vector.memset(v_sb[:, :, :, HD:HD + 1], 1.0)

            # PV accumulator: 2 banks, 4 q-chunk slots of 65 per bank.
            pv = pvpsp.tile([128, 2, 512], F32, name="pv", tag="pv")

            # ---- inputs for the projections (released with phb) ----
            xT_sb = phb.tile([128, KT, S], FP16)
            wq_sb = phb.tile([128, KT, 512], FP16)
            wk_sb = phb.tile([128, KT, 512], FP16)
            wv_sb = phb.tile([128, KT, 512], FP16)
            _dma_engines = [nc.sync, nc.gpsimd, nc.scalar]
            # coarse gather-DMAs keep the HWDGE queues short; the first K/Q
            # weight and x column-block gathers are split into kt-halves across
            # queues so the prologue's first accumulation pieces start early
            def half_gather(eng, dst, src, half):
                ks = slice(4 * half, 4 * half + 4)
                eng.dma_start(
                    out=dst[:, ks, :],
                    in_=src.rearrange("(a p) f -> p a f", p=128)[:, ks, :])

            half_gather(nc.sync, wk_sb, wk_d, 0)
            half_gather(nc.gpsimd, xT_sb[:, :, 0:512], xT_d[:, 0:512], 0)
            half_gather(nc.scalar, wq_sb, wq_d, 0)
            nc.scalar.dma_start(out=cos_sb, in_=cos_d)
            half_gather(nc.sync, wk_sb, wk_d, 1)
            half_gather(nc.gpsimd, xT_sb[:, :, 0:512], xT_d[:, 0:512], 1)
            nc.gpsimd.dma_start(out=sin_sb, in_=sin_d)
            half_gather(nc.scalar, wq_sb, wq_d, 1)
            nc.scalar.dma_start(out=neg_sb, in_=neg_d)
            nc.scalar.dma_start(out=idn_sb, in_=idn_d)
            nc.sync.dma_start(
                out=xT_sb[:, :, 512:1024],
                in_=xT_d[:, 512:1024].rearrange("(a p) f -> p a f", p=128))
            nc.gpsimd.dma_start(
                out=wv_sb[:], in_=wv_d.rearrange("(a p) f -> p a f", p=128))
            nc.sync.dma_start(
                out=xT_sb[:, :, 1024:1536],
                in_=xT_d[:, 1024:1536].rearrange("(a p) f -> p a f", p=128))
            nc.gpsimd.dma_start(
                out=xT_sb[:, :, 1536:2048],
                in_=xT_d[:, 1536:2048].rearrange("(a p) f -> p a f", p=128))
            nc.scalar.dma_start(
                out=wp_sb[:], in_=wp_d.rearrange("(a p) f -> p a f", p=128))

            def accum512(dst_view, lhsT_of_kt, rhs_of_kt, name, src_rearrange=None,
                         width=512, evac_act=False):
                """8-step k-accumulation into a [128, width] aux psum, evac'd to dst."""
                ps = auxp.tile([128, width], F32, tag="aux", name=name)
                for kt in range(KT):
                    nc.tensor.matmul(
                        ps[:], lhsT_of_kt(kt), rhs_of_kt(kt),
                        start=(kt == 0), stop=(kt == KT - 1),
                    )
                src = ps[:] if src_rearrange is None else ps[:].rearrange(*src_rearrange, d=HD)
                if evac_act:
                    nc.scalar.copy(dst_view, src)
                else:
                    nc.vector.tensor_copy(dst_view, src)

            def emit_v(sc, pair, evac_act=False):
                """V for seq chunk sc, heads 2*pair..2*pair+1 (one t-group)."""
                accum512(
                    v_sb[:, sc, 2 * pair:2 * pair + 2, 0:HD],
                    lambda kt, sc=sc: xT_sb[:, kt, sc * 128:(sc + 1) * 128],
                    lambda kt, pair=pair: wv_sb[:, kt, pair * 128:(pair + 1) * 128],
                    name=f"vps{sc}_{pair}",
                    src_rearrange=("p (h d) -> p h d",),
                    width=128,
                    evac_act=evac_act,
                )

            rope_raw = {}

            def emit_kq_quarter(t, which, qc, evac_act=False, piece=None):
                """piece=None: whole 8-step accumulation; piece=0..3: split
                into four 2-matmul sub-groups (finer filler granularity)."""
                w_sb = wk_sb if which == 0 else wq_sb
                if qc == 0 and piece in (None, 0):
                    rope_raw[(t, which)] = ropep.tile(
                        [128, S], FP16, tag="raw", bufs=2, name=f"raw{t}_{which}")
                raw = rope_raw[(t, which)]
                key = (t, which, qc)
                if piece in (None, 0):
                    kq_ps[key] = auxp.tile(
                        [128, 512], F32, tag="aux", name=f"kq{t}_{which}_{qc}")
                ps = kq_ps[key]
                kts = range(KT) if piece is None else range(4 * piece, 4 * piece + 4)
                for kt in kts:
                    nc.tensor.matmul(
                        ps[:],
                        w_sb[:, kt, t * 128:(t + 1) * 128],
                        xT_sb[:, kt, qc * 512:(qc + 1) * 512],
                        start=(kt == 0), stop=(kt == KT - 1),
                        skip_group_check=True,
                    )
                if piece in (None, 1):
                    kq_ps.pop(key)
                    dst = raw[:, qc * 512:(qc + 1) * 512]
                    if evac_act:
                        nc.scalar.copy(dst, ps[:])
                    else:
                        nc.vector.tensor_copy(dst, ps[:])

            def emit_rope(t, which, c0=0, c1=S, last=True, mul_pool=False):
                raw = rope_raw[(t, which)]
                if last:
                    rope_raw.pop((t, which))
                dst = kT if which == 0 else qT
                cs = slice(c0, c1)
                sw = ropep.tile([128, c1 - c0], FP16, tag="sw", bufs=3,
                                name=f"sw{t}_{which}_{c0}")
                eng = nc.gpsimd if mul_pool else nc.vector
                nc.vector.stream_shuffle(sw[:], raw[:, cs], SWAP_MASK)
                eng.tensor_mul(raw[:, cs], raw[:, cs], cos_sb[:, cs])
                eng.tensor_mul(sw[:], sw[:], sin_sb[:, cs])
                eng.tensor_add(dst[t][:, cs], raw[:, cs], sw[:])

            def emit_transpose(t, par, qh, yq, name, split_evac=False):
                """PE-transpose the normalized [q, hd] chunks back to [hd, q]."""
                rows = slice(64 * par, 64 * par + 64)
                tp = auxp.tile([64, 8, 128], FP16, tag="aux", name=f"tp{name}")
                for qc in range(8):
                    nc.tensor.transpose(tp[:, qc, :], yq[:, qc, :], idn_sb[:])
                    if split_evac and qc == 3:
                        nc.vector.tensor_copy(
                            yn[t][rows, qh * 1024:qh * 1024 + 512],
                            tp[:, 0:4, :].rearrange("p a b -> p (a b)"))
                if split_evac:
                    nc.vector.tensor_copy(
                        yn[t][rows, qh * 1024 + 512:(qh + 1) * 1024],
                        tp[:, 4:8, :].rearrange("p a b -> p (a b)"))
                else:
                    nc.vector.tensor_copy(
                        yn[t][rows, qh * 1024:(qh + 1) * 1024],
                        tp[:].rearrange("p a b -> p (a b)"))

            proj_osb = {}

            def emit_proj(pair, sc, evac_act=False, nn_only=None):
                """Project head-groups t=2*pair,2*pair+1 for seq chunk sc, summed.
                nn_only splits the two output halves into separate calls."""
                key = (pair, sc)
                if nn_only in (None, 0):
                    proj_osb[key] = outp.tile(
                        [128, DM], F32, tag="osb", name=f"osb{pair}_{sc}")
                osb = proj_osb[key]
                nns = range(2) if nn_only is None else (nn_only,)
                for nn in nns:
                    pp = auxp.tile([128, 512], F32, tag="aux", name=f"pp{pair}_{sc}_{nn}")
                    for i, t in enumerate((2 * pair, 2 * pair + 1)):
                        nc.tensor.matmul(
                            pp[:],
                            yn[t][:, sc * 128:(sc + 1) * 128],
                            wp_sb[:, t, nn * 512:(nn + 1) * 512],
                            start=(i == 0),
                            stop=(i == 1),
                        )
                    use_act = (nn == 1) if evac_act == "alt" else evac_act
                    if use_act:
                        nc.scalar.copy(osb[:, nn * 512:(nn + 1) * 512], pp[:])
                    else:
                        nc.vector.tensor_copy(osb[:, nn * 512:(nn + 1) * 512], pp[:])
                    if evac_act == "alt":
                        _dma_engines[nn % 2].dma_start(
                            out=out_d[pair][sc * 128:(sc + 1) * 128,
                                            nn * 512:(nn + 1) * 512],
                            in_=osb[:, nn * 512:(nn + 1) * 512])
                if evac_act == "alt":
                    if nn_only in (None, 1):
                        proj_osb.pop(key)
                elif nn_only in (None, 1):
                    proj_osb.pop(key)
                    _dma_engines[sc % 2].dma_start(
                        out=out_d[pair][sc * 128:(sc + 1) * 128, :], in_=osb[:])

            # Fillers are (pe_cost_ns, fn, cont); drip() spends a fixed
            # PE-time budget per call so filler work spreads evenly across the
            # slots instead of bunching (which would delay the scores feeding
            # ACT). cont=True marks the second half of a split accumulation
            # group: FIFO order keeps any other aux-pool user behind it.
            fillers = []
            debt = [0.0]
            kq_ps = {}

            def drip(budget):
                debt[0] = min(debt[0] + budget, 1200.0)
                while fillers and debt[0] >= fillers[0][0]:
                    cost, fn, _ = fillers.pop(0)
                    debt[0] -= cost
                    fn()

            def push_front(cost, fn):
                i = 0
                while i < len(fillers) and fillers[i][2]:
                    i += 1
                fillers.insert(i, (cost, fn, False))

            KQH_COST, V_COST, PROJH_COST, TP_COST = 440.0, 440.0, 440.0, 460.0

            # ---- prologue: the minimum gating the first exp: the low half of
            # K and Q for tile 0 (covers scores kc<8, q-half 0); the rest
            # drips. Evacs ride the (idle) ACT engine - DVE is busy with rope ----
            for qc in range(2):
                for p in range(2):
                    emit_kq_quarter(0, 0, qc, evac_act=True, piece=p)
            emit_rope(0, 0, 0, 1024, last=False)
            for qc in range(2):
                for p in range(2):
                    emit_kq_quarter(0, 1, qc, evac_act=True, piece=p)
            emit_rope(0, 1, 0, 1024, last=False)

            def pv_slot(qc, w=65):
                return pv[:, qc // 4, (qc % 4) * 65:(qc % 4) * 65 + w]

            def emit_scores(t, rows, qh, kc, name):
                sps = spsp.tile([128, 1024], F32, tag="s", name=f"s{name}_{kc}")
                for qg in range(2):
                    q0 = qh * 1024 + qg * 512
                    nc.tensor.matmul(
                        sps[:, qg * 512:(qg + 1) * 512],
                        kT[t][rows, kc * 128:(kc + 1) * 128],
                        qT[t][rows, q0:q0 + 512],
                        start=True,
                        stop=True,
                    )
                return sps

            def kq_chunks(t):
                """Chunks of filler entries: each kq quarter is 2 cont-pieces;
                rope halves follow the pair of quarters they depend on."""
                out = []
                for which in (0, 1):
                    for qc in range(4):
                        out.append([
                            (KQH_COST,
                             lambda w=which, qc=qc, t=t, p=p: emit_kq_quarter(
                                 t, w, qc, piece=p),
                             p > 0)
                            for p in range(2)
                        ])
                        if qc == 1:
                            out.append([(0.0,
                                         lambda w=which, t=t: emit_rope(
                                             t, w, 0, 1024, last=False,
                                             mul_pool=True),
                                         False)])
                        elif qc == 3:
                            out.append([(0.0,
                                         lambda w=which, t=t: emit_rope(
                                             t, w, 1024, S, mul_pool=True),
                                         False)])
                return out

            def proj_entries(pair, scs):
                out = []
                for sc in scs:
                    out.append((PROJH_COST,
                                lambda pair=pair, sc=sc: emit_proj(pair, sc, nn_only=0),
                                False))
                    out.append((PROJH_COST,
                                lambda pair=pair, sc=sc: emit_proj(pair, sc, nn_only=1),
                                False))
                return out

            def interleave(a, b):
                """a: list of entry-chunks; b: list of single entries."""
                out = []
                for i in range(max(len(a), len(b))):
                    if i < len(b):
                        out.append(b[i])
                    if i < len(a):
                        out.extend(a[i])
                return out

            def pv8g(kc, pt, hh):
                for qc in range(8):
                    nc.tensor.matmul(
                        pv_slot(qc),
                        pt[:, qc * 128:(qc + 1) * 128],
                        v_sb[:, kc, hh, :],
                        start=False,
                        stop=True,
                        skip_group_check=True,
                    )

            def emit_normalize(ot, opar, oqh, oh, last):
                """Reciprocal + per-partition scale in [q, hd] layout, then the
                transpose back to [hd, q] (queued as a filler unless last)."""
                rz = rzp.tile([128, 8], F32, tag="rz", name=f"rz{oh}_{oqh}")
                pvap = pv[:]
                zs = bass.AP(
                    tensor=pvap.tensor,
                    offset=pvap.offset + 64,
                    ap=[list(pvap.ap[0]), [512, 2], [65, 4]],
                )
                nc.vector.reciprocal(rz[:].rearrange("p (a b) -> p a b", a=2), zs)
                yq = yqp.tile([128, 8, HD], FP16, tag="yq", name=f"yq{oh}_{oqh}")
                for qc in range(8):
                    if last and qc % 2 == 1:
                        # tail only: the exp stream is over, ACT is free -
                        # halve the serial normalize chain on the critical path
                        nc.scalar.mul(yq[:, qc, :], pv_slot(qc, HD),
                                      rz[:, qc:qc + 1])
                    else:
                        nc.vector.tensor_scalar_mul(
                            yq[:, qc, :], pv_slot(qc, HD), rz[:, qc:qc + 1])
                if last:
                    emit_transpose(ot, opar, oqh, yq, f"{oh}_{oqh}",
                                   split_evac=True)
                else:
                    push_front(TP_COST,
                               lambda t=ot, par=opar, qh=oqh, yq=yq, h=oh:
                               emit_transpose(t, par, qh, yq, f"{h}_{qh}"))

            # ---- attention slots: (t, qh, par) ordering so that by the time
            # the last head's qh=1 half runs, proj for the earlier halves can
            # already drip. V for pair p and K/Q for tile t arrive one block
            # ahead of first use, spread across the preceding block's slots. ----
            slots = [(t, qh, par) for t in range(4) for qh in range(2)
                     for par in range(2)]
            handoff = {"sps": None}
            carry = {"ptq": [], "h": 0, "t": 0, "par": 0, "qh": 0}
            for t in range(4):
                for qh in range(2):
                    for par in range(2):
                        h = 2 * t + par
                        slot = 4 * t + 2 * qh + par
                        if slot == 0:
                            for w in (0, 1):
                                for qc in range(2, 4):
                                    for p in range(2):
                                        fillers.append(
                                            (KQH_COST,
                                             lambda w=w, qc=qc, p=p: emit_kq_quarter(
                                                 0, w, qc, piece=p),
                                             p > 0))
                                    fillers.append(
                                        (0.0,
                                         lambda w=w, qc=qc: emit_rope(
                                             0, w, qc * 512, (qc + 1) * 512,
                                             last=(qc == 3)),
                                         False))
                            fillers += interleave(
                                kq_chunks(1),
                                [(V_COST, lambda sc=sc: emit_v(sc, 1), False)
                                 for sc in range(SC)])
                        elif slot == 4:
                            fillers += interleave(
                                kq_chunks(2),
                                [(V_COST, lambda sc=sc: emit_v(sc, 2), False)
                                 for sc in range(SC)])
                        elif slot == 8:
                            fillers += interleave(
                                kq_chunks(3),
                                [(V_COST, lambda sc=sc: emit_v(sc, 3), False)
                                 for sc in range(SC)])
                            # t01 projections: heads 0-3 fully transposed by now
                            # (the slot-7 transposes are at the queue front)
                            fillers += proj_entries(0, range(SC))
                        elif slot == 14:
                            # t23 projections for the qh=0 seq half
                            fillers += proj_entries(1, range(8))

                        rows = slice(64 * par, 64 * par + 64)
                        if slot == 0:
                            # zero the PV accumulator regions (start=False
                            # accumulation; GPSIMD cannot touch PSUM, so DVE)
                            nc.vector.memset(pv[:, :, 0:260], 0.0)
                        # software pipeline: scores(kc+1) and any filler are
                        # issued before pv(kc), which lags its exp by one strip
                        # so it never head-of-line-blocks the PE queue. The
                        # first scores of this slot were already emitted at the
                        # tail of the previous slot (before its last pv8).
                        if handoff["sps"] is not None:
                            sps = handoff["sps"]
                            handoff["sps"] = None
                        else:
                            sps = emit_scores(t, rows, qh, 0, f"{h}_{qh}")
                            emit_v(0, 0, evac_act=True)
                            emit_v(1, 0, evac_act=True)
                        drip(800)

                        ptq = []
                        for kc in range(SC):
                            pt = ptp.tile([128, 1024], FP16, tag="pt", name=f"pt{h}_{qh}_{kc}")
                            nc.scalar.activation(pt[:], sps[:], Exp, scale=SCALE)
                            if kc // 8 == qh:
                                c0 = kc * 128 - qh * 1024
                                nc.vector.tensor_mul(
                                    pt[:, c0:c0 + 128], pt[:, c0:c0 + 128], neg_sb[:])
                            if kc + 1 < SC:
                                sps = emit_scores(t, rows, qh, kc + 1, f"{h}_{qh}")
                            if slot == 0 and kc + 2 < SC:
                                emit_v(kc + 2, 0)
                                drip(180)
                            else:
                                drip(270 if slot < 4 else 250)
                            # drain the previous slot's carried PV groups (2 per
                            # iteration), then its normalize + the accumulator
                            # re-zero; our own pv8s start at iteration 4 so the
                            # memset lands with two iterations of slack
                            if carry["ptq"]:
                                for _ in range(2):
                                    if carry["ptq"]:
                                        okc, opt = carry["ptq"].pop(0)
                                        pv8g(okc, opt, carry["h"])
                                if not carry["ptq"]:
                                    emit_normalize(carry["t"], carry["par"],
                                                   carry["qh"], carry["h"], False)
                                    nc.vector.memset(pv[:, :, 0:260], 0.0)
                            else:
                                ptq.append((kc, pt))
                                if len(ptq) > 4:
                                    okc, opt = ptq.pop(0)
                                    pv8g(okc, opt, h)
                                continue
                            ptq.append((kc, pt))
                        # pre-emit the next slot's first scores, then either
                        # carry the PV flush into the next slot or (last slot)
                        # flush and normalize here
                        if slot < 15:
                            nt, nqh, npar = slots[slot + 1]
                            handoff["sps"] = emit_scores(
                                nt, slice(64 * npar, 64 * npar + 64), nqh, 0,
                                f"{2 * nt + npar}_{nqh}")
                            carry.update(ptq=ptq, h=h, t=t, par=par, qh=qh)
                        else:
                            for okc, opt in ptq:
                                pv8g(okc, opt, h)
                            emit_normalize(t, par, qh, h, True)

            while fillers:
                fillers.pop(0)[1]()
            for sc in range(8, SC):
                emit_proj(1, sc, evac_act="alt")

    nc.compile()
    return nc


def _host_tables():
    theta = 1.0 / (10000.0 ** (np.arange(0, HD, 2, dtype=np.float32) / HD))
    ang = np.arange(S, dtype=np.float32)[:, None] * theta[None, :]  # [S, 32]
    cos = np.repeat(np.cos(ang).T, 2, axis=0)  # [64, S]
    sin_ = np.empty((HD, S), np.float32)
    sin_[0::2] = -np.sin(ang).T
    sin_[1::2] = np.sin(ang).T
    cosb = np.concatenate([cos, cos], axis=0).astype(np.float16)  # [128, S]
    sinb = np.concatenate([sin_, sin_], axis=0).astype(np.float16)
    negi = (1.0 - np.eye(128, dtype=np.float32)).astype(np.float16)
    ident = np.eye(128, dtype=np.float32).astype(np.float16)
    return cosb, sinb, negi, ident


def _in_maps(x, Wqkv, Wproj):
    cosb, sinb, negi, ident = _host_tables()
    maps = []
    for core in range(N_CORES):
        b, hh = divmod(core, 2)
        c0 = hh * 512
        maps.append(
            {
                "xT": np.ascontiguousarray(x[b].T).astype(np.float16),
                "wq": np.ascontiguousarray(Wqkv[:, c0:c0 + 512]).astype(np.float16),
                "wk": np.ascontiguousarray(Wqkv[:, DM + c0:DM + c0 + 512]).astype(np.float16),
                "wv": np.ascontiguousarray(Wqkv[:, 2 * DM + c0:2 * DM + c0 + 512]).astype(np.float16),
                "wp": np.ascontiguousarray(Wproj[c0:c0 + 512, :]).astype(np.float16),
                "cosb": cosb,
                "sinb": sinb,
                "negi": negi,
                "ident": ident,
            }
        )
    return maps


def kernel(x, Wqkv, Wproj):
    if "nc" not in _CACHE:
        _CACHE["nc"] = _build_nc()
    nc = _CACHE["nc"]

    x = np.asarray(x)
    Wqkv = np.asarray(Wqkv)
    Wproj = np.asarray(Wproj)

    res = run_bass_kernel_spmd(nc, _in_maps(x, Wqkv, Wproj), core_ids=list(range(N_CORES)))
    out = np.empty((B, S, DM), np.float32)
    for b in range(B):
        acc = None
        for core in (2 * b, 2 * b + 1):
            for p in range(2):
                part = res.results[core][f"out{p}"]
                acc = part if acc is None else acc + part
        out[b] = acc
    return out
